# revision 13
# baseline (speedup 1.0000x reference)
"""CharRNN (2-layer miLSTM + big logits GEMM) Trainium2 kernel.

Sharding: data-parallel over batch across 8 cores (4 sequences each).
Each core runs the full T=128 recurrence for its 4 sequences and then
computes logits for its own 512 tokens over the FULL vocab (no
collectives). Host concatenates the 8 (512, 32000) shards.

On-device layout is "transposed": features live on partitions, batch on
the free dim, so per-feature vectors (wi/wf/wo, beta2, bias) become
per-partition scalars and h feeds matmuls without transposes.

Host-side weight prep folds alpha/beta1 into Wx (column scales), adds
FORGET_BIAS into the f-gate bias, and permutes the gate order to
[i|f|j|o] so the i/f sigmoid runs as one activation op.
"""

import numpy as np
from contextlib import ExitStack

V, E, L, B, T = 32000, 128, 2, 32, 128
G = 4 * E
P = 128
NCORES = 8
BL = B // NCORES          # 4 sequences per core
NTOK = BL * T             # 512 tokens per core
FORGET_BIAS = 1.0
NB = 4                    # pipeline blocks (32 steps / 128 tokens each)
SPB = T // NB             # steps per block = 32
TPB = SPB * BL            # tokens per block = 128
NT_FULL = V // 512        # 62 full 512-wide logits n-tiles
NT_LAST = V - NT_FULL * 512  # 256
N_NT = NT_FULL + 1        # 63 n-tiles

_cache = {}


def _build(use_smax_bias, stage=3):
    import concourse.tile as tile
    import concourse.mybir as mybir
    from concourse import bacc
    from concourse.bass import IndirectOffsetOnAxis
    from concourse.masks import make_identity

    dt = mybir.dt
    AF = mybir.ActivationFunctionType
    OP = mybir.AluOpType

    nc = bacc.Bacc("TRN2", target_bir_lowering=False, debug=False,
                   num_devices=NCORES)

    ids_d = nc.dram_tensor("ids", (P, BL), dt.int32, kind="ExternalInput")
    emb_d = nc.dram_tensor("emb", (V, E), dt.float32, kind="ExternalInput")
    wxa_d = nc.dram_tensor("wxa", (P, L, G), dt.float32, kind="ExternalInput")
    wxc_d = nc.dram_tensor("wxc", (P, L, G), dt.float32, kind="ExternalInput")
    wh_d = nc.dram_tensor("wh", (P, L, G), dt.float32, kind="ExternalInput")
    b2t_d = nc.dram_tensor("b2t", (P, L, 4), dt.float32, kind="ExternalInput")
    bft_d = nc.dram_tensor("bft", (P, L, 4), dt.float32, kind="ExternalInput")
    pep_d = nc.dram_tensor("pep", (P, L, 3), dt.float32, kind="ExternalInput")
    swt_d = nc.dram_tensor("swt", (P, V), dt.float32, kind="ExternalInput")
    if use_smax_bias:
        smb_d = nc.dram_tensor("smb", (1, V), dt.float32, kind="ExternalInput")
    # rows of out are in device token order (t*BL + s); host un-permutes
    out_d = nc.dram_tensor("out", (NTOK, V), dt.float32, kind="ExternalOutput")

    with tile.TileContext(nc) as tc, ExitStack() as ctx:
        singles = ctx.enter_context(tc.tile_pool(name="singles", bufs=1))
        big = ctx.enter_context(tc.tile_pool(name="big", bufs=1))
        stage_p = ctx.enter_context(tc.tile_pool(name="stage", bufs=6))
        rec = ctx.enter_context(tc.tile_pool(name="rec", bufs=3))
        cpool = ctx.enter_context(tc.tile_pool(name="cpool", bufs=3))
        ps_big = ctx.enter_context(
            tc.tile_pool(name="ps_big", bufs=2, space="PSUM"))
        ps_g0 = ctx.enter_context(
            tc.tile_pool(name="ps_g0", bufs=2, space="PSUM"))
        ps_g1 = ctx.enter_context(
            tc.tile_pool(name="ps_g1", bufs=2, space="PSUM"))
        ps_log = ctx.enter_context(
            tc.tile_pool(name="ps_log", bufs=2, space="PSUM"))

        # ---- static inputs -> SBUF ----
        ids_sb = singles.tile([P, BL], dt.int32)
        nc.sync.dma_start(out=ids_sb[:, :], in_=ids_d[:, :])
        wxa_sb = singles.tile([P, L, G], dt.float32)
        nc.sync.dma_start(out=wxa_sb[:, :, :], in_=wxa_d[:, :, :])
        wxc_sb = singles.tile([P, L, G], dt.float32)
        nc.sync.dma_start(out=wxc_sb[:, :, :], in_=wxc_d[:, :, :])
        wh_sb = singles.tile([P, L, G], dt.float32)
        nc.sync.dma_start(out=wh_sb[:, :, :], in_=wh_d[:, :, :])
        b2t_sb = singles.tile([P, L, 4], dt.float32)
        nc.sync.dma_start(out=b2t_sb[:, :, :], in_=b2t_d[:, :, :])
        bft_sb = singles.tile([P, L, 4], dt.float32)
        nc.sync.dma_start(out=bft_sb[:, :, :], in_=bft_d[:, :, :])
        pep_sb = singles.tile([P, L, 3], dt.float32)
        nc.sync.dma_start(out=pep_sb[:, :, :], in_=pep_d[:, :, :])
        swt_sb = singles.tile([P, V], dt.float32)
        # split the 16MB load into 8 chunks so it spreads across queues
        for q in range(8):
            nc.sync.dma_start(out=swt_sb[:, q * 4000:(q + 1) * 4000],
                              in_=swt_d[:, q * 4000:(q + 1) * 4000])
        if use_smax_bias:
            smb_sb = singles.tile([1, V], dt.float32)
            nc.sync.dma_start(out=smb_sb[:, :], in_=smb_d[:, :])
            ones1 = singles.tile([1, P], dt.float32)
            nc.vector.memset(ones1[:, :], 1.0)

        ident = singles.tile([P, P], dt.float32)
        make_identity(nc, ident[:, :])

        zeros4 = singles.tile([P, BL], dt.float32)
        nc.vector.memset(zeros4[:, :], 0.0)

        # ---- embedding gather (tokens on partitions) + transpose ----
        x_sb = singles.tile([P, BL, E], dt.float32)
        for m in range(BL):
            nc.gpsimd.indirect_dma_start(
                out=x_sb[:, m, :], out_offset=None,
                in_=emb_d[:, :],
                in_offset=IndirectOffsetOnAxis(ap=ids_sb[:, m:m + 1], axis=0),
            )
        xT = singles.tile([P, NTOK], dt.float32)
        for m in range(BL):
            pst = ps_big.tile([P, P], dt.float32, tag="psac")
            nc.tensor.transpose(pst[:, :], x_sb[:, m, :], ident[:, :])
            nc.scalar.copy(xT[:, m * P:(m + 1) * P], pst[:, :])

        # ---- A/C buffers and hidden-state buffers ----
        a_sb = [big.tile([P, 4, NTOK], dt.float32, name=f"a{l}", tag=f"a{l}")
                for l in range(L)]
        c_sb = [big.tile([P, 4, NTOK], dt.float32, name=f"cc{l}", tag=f"cc{l}")
                for l in range(L)]
        hT = [big.tile([P, NTOK], dt.float32, name=f"h{l}", tag=f"h{l}")
              for l in range(L)]

        def emit_ac_block(l, src, j):
            # A[:, k, jblk] = WxA[l,k].T @ src_blk + beta2 ; C likewise + bias
            blk = slice(j * TPB, (j + 1) * TPB)
            for k in range(4):
                psa = ps_big.tile([P, TPB], dt.float32, tag="psac")
                nc.tensor.matmul(psa[:, :], wxa_sb[:, l, k * P:(k + 1) * P],
                                 src[:, blk])
                nc.scalar.activation(a_sb[l][:, k, blk], psa[:, :],
                                     AF.Identity, bias=b2t_sb[:, l, k:k + 1])
                psc = ps_big.tile([P, TPB], dt.float32, tag="psac")
                nc.tensor.matmul(psc[:, :], wxc_sb[:, l, k * P:(k + 1) * P],
                                 src[:, blk])
                nc.vector.tensor_scalar_add(c_sb[l][:, k, blk], psc[:, :],
                                            bft_sb[:, l, k:k + 1])

        # per-layer recurrence state (APs)
        c_prev = [zeros4[:, :], zeros4[:, :]]
        h_prev = [zeros4[:, :], zeros4[:, :]]
        ps_g = [ps_g0, ps_g1]

        def emit_step(l, t):
            tb = slice(t * BL, (t + 1) * BL)
            psg = ps_g[l].tile([P, 4, BL], dt.float32)
            for k in range(4):
                nc.tensor.matmul(psg[:, k, :], wh_sb[:, l, k * P:(k + 1) * P],
                                 h_prev[l], start=(k == 0), stop=(k == 3),
                                 skip_group_check=True)
            g = rec.tile([P, 4, BL], dt.float32, tag=f"g{l}")
            nc.vector.tensor_tensor(g[:, :, :], psg[:, :, :],
                                    a_sb[l][:, :, tb], op=OP.mult)
            nc.vector.tensor_tensor(g[:, :, :], g[:, :, :],
                                    c_sb[l][:, :, tb], op=OP.add)
            cp = c_prev[l]
            if2 = rec.tile([P, 2, BL], dt.float32, tag=f"if{l}")
            nc.vector.scalar_tensor_tensor(
                if2[:, 0, :], cp, pep_sb[:, l, 0:1], g[:, 0, :],
                op0=OP.mult, op1=OP.add)
            nc.vector.scalar_tensor_tensor(
                if2[:, 1, :], cp, pep_sb[:, l, 1:2], g[:, 1, :],
                op0=OP.mult, op1=OP.add)
            sif = rec.tile([P, 2, BL], dt.float32, tag=f"sif{l}")
            nc.scalar.activation(sif[:, :, :], if2[:, :, :], AF.Sigmoid)
            tj = rec.tile([P, BL], dt.float32, tag=f"tj{l}")
            nc.scalar.activation(tj[:, :], g[:, 2, :], AF.Tanh)
            u = rec.tile([P, BL], dt.float32, tag=f"u{l}")
            nc.gpsimd.tensor_tensor(u[:, :], sif[:, 0, :], tj[:, :],
                                    op=OP.mult)
            v = rec.tile([P, BL], dt.float32, tag=f"v{l}")
            nc.vector.tensor_tensor(v[:, :], sif[:, 1, :], cp, op=OP.mult)
            cn = cpool.tile([P, BL], dt.float32, tag=f"c{l}")
            nc.vector.tensor_tensor(cn[:, :], u[:, :], v[:, :], op=OP.add)
            o2 = rec.tile([P, BL], dt.float32, tag=f"o2{l}")
            nc.vector.scalar_tensor_tensor(
                o2[:, :], cn[:, :], pep_sb[:, l, 2:3], g[:, 3, :],
                op0=OP.mult, op1=OP.add)
            so = rec.tile([P, BL], dt.float32, tag=f"so{l}")
            nc.scalar.activation(so[:, :], o2[:, :], AF.Sigmoid)
            tc_ = rec.tile([P, BL], dt.float32, tag=f"tc{l}")
            nc.scalar.activation(tc_[:, :], cn[:, :], AF.Tanh)
            nc.gpsimd.tensor_tensor(hT[l][:, tb], so[:, :], tc_[:, :],
                                    op=OP.mult)
            c_prev[l] = cn[:, :]
            h_prev[l] = hT[l][:, tb]

        def emit_logits_ntile(k, n, eng):
            n0 = n * 512
            nn = 512 if n < NT_FULL else NT_LAST
            ps = ps_log.tile([P, 512], dt.float32)
            nc.tensor.matmul(ps[:, 0:nn], hT[1][:, k * TPB:(k + 1) * TPB],
                             swt_sb[:, n0:n0 + nn],
                             start=True, stop=not use_smax_bias)
            if use_smax_bias:
                nc.tensor.matmul(ps[:, 0:nn], ones1[:, :],
                                 smb_sb[:, n0:n0 + nn], start=False, stop=True)
            st = stage_p.tile([P, 512], dt.float32)
            if eng == 0:
                nc.vector.tensor_copy(st[:, 0:nn], ps[:, 0:nn])
            else:
                nc.scalar.copy(st[:, 0:nn], ps[:, 0:nn])
            nc.sync.dma_start(
                out=out_d[k * TPB:(k + 1) * TPB, n0:n0 + nn],
                in_=st[:, 0:nn])

        # layer-0 A/C for all tokens (x fully available)
        for j in range(NB):
            emit_ac_block(0, xT, j)

        # ---- pipelined recurrence + logits ----
        pending = []   # (mtile, ntile) logits work
        ne = 0
        for jj in range(NB + 1):
            for i in range(SPB):
                if jj < NB and stage >= 1:
                    emit_step(0, jj * SPB + i)
                if jj >= 1 and stage >= 2:
                    emit_step(1, (jj - 1) * SPB + i)
                for _ in range(2):
                    if ne < len(pending):
                        k, n = pending[ne]
                        emit_logits_ntile(k, n, ne % 2)
                        ne += 1
            if jj < NB and stage >= 2:
                emit_ac_block(1, hT[0], jj)
            if jj >= 1 and stage >= 3:
                pending.extend(((jj - 1, n) for n in range(N_NT)))
        while ne < len(pending):
            k, n = pending[ne]
            emit_logits_ntile(k, n, ne % 2)
            ne += 1

    nc.compile()
    return nc


def _prep_inputs(input_data, embedding, Wx, Wh, alpha, beta1, beta2, bias,
                 wi, wf, wo, softmax_w, softmax_b):
    f32 = np.float32
    input_data = np.asarray(input_data, np.int32)
    embedding = np.ascontiguousarray(np.asarray(embedding, f32))
    Wx = np.asarray(Wx, f32)
    Wh = np.asarray(Wh, f32)
    alpha = np.asarray(alpha, f32)
    beta1 = np.asarray(beta1, f32)
    beta2 = np.asarray(beta2, f32)
    bias = np.asarray(bias, f32)
    wi = np.asarray(wi, f32)
    wf = np.asarray(wf, f32)
    wo = np.asarray(wo, f32)
    softmax_w = np.asarray(softmax_w, f32)
    softmax_b = np.asarray(softmax_b, f32)

    gperm = [0, 2, 1, 3]   # reference order i,j,f,o -> device order i,f,j,o

    def permG(a):           # (..., G) -> gate chunks reordered
        r = a.reshape(*a.shape[:-1], 4, E)
        return np.ascontiguousarray(r[..., gperm, :].reshape(*a.shape))

    WxA = permG(Wx * alpha[:, None, :])          # (L, E, G)
    WxC = permG(Wx * beta1[:, None, :])
    Whp = permG(Wh)
    b2p = permG(beta2)                           # (L, G)
    bp = permG(bias).copy()
    bp[:, E:2 * E] += FORGET_BIAS                # f-chunk

    def to_elg(a):          # (L, E, G) -> (E, L, G) contiguous
        return np.ascontiguousarray(np.transpose(a, (1, 0, 2)))

    def to_plk(a):          # (L, G) -> (P, L, 4): partition=feature, per-gate
        return np.ascontiguousarray(
            np.transpose(a.reshape(L, 4, E), (2, 0, 1)))

    pep = np.ascontiguousarray(
        np.transpose(np.stack([wi, wf, wo], axis=1), (2, 0, 1)))  # (E, L, 3)

    swt = np.ascontiguousarray(softmax_w.T)      # (E, V)
    use_smax_bias = bool(np.any(softmax_b))

    common = {
        "emb": embedding,
        "wxa": to_elg(WxA), "wxc": to_elg(WxC), "wh": to_elg(Whp),
        "b2t": to_plk(b2p), "bft": to_plk(bp), "pep": pep,
        "swt": swt,
    }
    if use_smax_bias:
        common["smb"] = softmax_b.reshape(1, V)

    tok = np.arange(NTOK)
    tt_, ss_ = tok // BL, tok % BL
    in_maps = []
    for c in range(NCORES):
        flat = input_data[BL * c + ss_, tt_]               # token order t*BL+s
        ids_pm = np.ascontiguousarray(flat.reshape(BL, P).T.astype(np.int32))
        in_maps.append({"ids": ids_pm, **common})
    return in_maps, use_smax_bias


def _run(in_maps, use_smax_bias, trace=False, tmpdir=None):
    from concourse.bass_utils import run_bass_kernel_spmd
    key = use_smax_bias
    if key not in _cache:
        _cache[key] = _build(use_smax_bias)
    nc = _cache[key]
    return run_bass_kernel_spmd(nc, in_maps, core_ids=list(range(NCORES)),
                                trace=trace, tmpdir=tmpdir)


def kernel(**inputs):
    in_maps, use_smax_bias = _prep_inputs(**inputs)
    res = _run(in_maps, use_smax_bias, trace=False)
    # device rows are token order (t*BL + s); reference rows are s*T + t
    tok = np.arange(NTOK)
    row = (tok % BL) * T + tok // BL          # device row tok -> output row
    out = np.empty((B * T, V), np.float32)
    for c in range(NCORES):
        out[c * NTOK + row] = res.results[c]["out"]
    return out


# revision 15
# speedup vs baseline: 1.5236x; 1.5236x over previous
"""CharRNN (2-layer miLSTM + big logits GEMM) Trainium2 kernel.

Sharding: data-parallel over batch across 8 cores (4 sequences each).
Each core runs the full T=128 recurrence for its 4 sequences and then
computes logits for its own 512 tokens over the FULL vocab (no
collectives). Host concatenates the 8 (512, 32000) shards.

On-device layout is "transposed": features live on partitions, batch on
the free dim, so per-feature vectors (wi/wf/wo, beta2, bias) become
per-partition scalars and h feeds matmuls without transposes.

Host-side weight prep folds alpha/beta1 into Wx (column scales), adds
FORGET_BIAS into the f-gate bias, and permutes the gate order to
[i|f|j|o] so the i/f sigmoid runs as one activation op.
"""

import numpy as np
from contextlib import ExitStack

V, E, L, B, T = 32000, 128, 2, 32, 128
G = 4 * E
P = 128
NCORES = 8
BL = B // NCORES          # 4 sequences per core
NTOK = BL * T             # 512 tokens per core
FORGET_BIAS = 1.0
NB = 4                    # pipeline blocks (32 steps / 128 tokens each)
SPB = T // NB             # steps per block = 32
TPB = SPB * BL            # tokens per block = 128
NT_FULL = V // 512        # 62 full 512-wide logits n-tiles
NT_LAST = V - NT_FULL * 512  # 256
N_NT = NT_FULL + 1        # 63 n-tiles

_cache = {}


def _build(use_smax_bias, stage=3):
    import concourse.tile as tile
    import concourse.mybir as mybir
    from concourse import bacc
    from concourse.bass import IndirectOffsetOnAxis
    from concourse.masks import make_identity

    dt = mybir.dt
    AF = mybir.ActivationFunctionType
    OP = mybir.AluOpType

    nc = bacc.Bacc("TRN2", target_bir_lowering=False, debug=False,
                   num_devices=NCORES)

    ids_d = nc.dram_tensor("ids", (P, BL), dt.int32, kind="ExternalInput")
    emb_d = nc.dram_tensor("emb", (V, E), dt.float32, kind="ExternalInput")
    wxa_d = nc.dram_tensor("wxa", (P, L, G), dt.bfloat16, kind="ExternalInput")
    wxc_d = nc.dram_tensor("wxc", (P, L, G), dt.bfloat16, kind="ExternalInput")
    wh_d = nc.dram_tensor("wh", (P, L, G), dt.bfloat16, kind="ExternalInput")
    b2t_d = nc.dram_tensor("b2t", (P, L, 4), dt.float32, kind="ExternalInput")
    bft_d = nc.dram_tensor("bft", (P, L, 4), dt.float32, kind="ExternalInput")
    pep_d = nc.dram_tensor("pep", (P, L, 3), dt.float32, kind="ExternalInput")
    swt_d = nc.dram_tensor("swt", (P, V), dt.bfloat16, kind="ExternalInput")
    if use_smax_bias:
        smb_d = nc.dram_tensor("smb", (1, V), dt.float32, kind="ExternalInput")
    # rows of out are in device token order (t*BL + s); host un-permutes
    out_d = nc.dram_tensor("out", (NTOK, V), dt.float32, kind="ExternalOutput")

    with tile.TileContext(nc) as tc, ExitStack() as ctx:
        singles = ctx.enter_context(tc.tile_pool(name="singles", bufs=1))
        big = ctx.enter_context(tc.tile_pool(name="big", bufs=1))
        stage_p = ctx.enter_context(tc.tile_pool(name="stage", bufs=6))
        rec = ctx.enter_context(tc.tile_pool(name="rec", bufs=3))
        cpool = ctx.enter_context(tc.tile_pool(name="cpool", bufs=3))
        ps_big = ctx.enter_context(
            tc.tile_pool(name="ps_big", bufs=2, space="PSUM"))
        ps_g0 = ctx.enter_context(
            tc.tile_pool(name="ps_g0", bufs=2, space="PSUM"))
        ps_g1 = ctx.enter_context(
            tc.tile_pool(name="ps_g1", bufs=2, space="PSUM"))
        ps_log = ctx.enter_context(
            tc.tile_pool(name="ps_log", bufs=2, space="PSUM"))

        # ---- static inputs -> SBUF ----
        ids_sb = singles.tile([P, BL], dt.int32)
        nc.sync.dma_start(out=ids_sb[:, :], in_=ids_d[:, :])
        wxa_sb = singles.tile([P, L, G], dt.bfloat16)
        nc.sync.dma_start(out=wxa_sb[:, :, :], in_=wxa_d[:, :, :])
        wxc_sb = singles.tile([P, L, G], dt.bfloat16)
        nc.sync.dma_start(out=wxc_sb[:, :, :], in_=wxc_d[:, :, :])
        wh_sb = singles.tile([P, L, G], dt.bfloat16)
        nc.sync.dma_start(out=wh_sb[:, :, :], in_=wh_d[:, :, :])
        b2t_sb = singles.tile([P, L, 4], dt.float32)
        nc.sync.dma_start(out=b2t_sb[:, :, :], in_=b2t_d[:, :, :])
        bft_sb = singles.tile([P, L, 4], dt.float32)
        nc.sync.dma_start(out=bft_sb[:, :, :], in_=bft_d[:, :, :])
        pep_sb = singles.tile([P, L, 3], dt.float32)
        nc.sync.dma_start(out=pep_sb[:, :, :], in_=pep_d[:, :, :])
        swt_sb = singles.tile([P, V], dt.bfloat16)
        # split the 16MB load into 8 chunks so it spreads across queues
        for q in range(8):
            nc.sync.dma_start(out=swt_sb[:, q * 4000:(q + 1) * 4000],
                              in_=swt_d[:, q * 4000:(q + 1) * 4000])
        if use_smax_bias:
            smb_sb = singles.tile([1, V], dt.float32)
            nc.sync.dma_start(out=smb_sb[:, :], in_=smb_d[:, :])
            ones1 = singles.tile([1, P], dt.float32)
            nc.vector.memset(ones1[:, :], 1.0)

        ident = singles.tile([P, P], dt.float32)
        make_identity(nc, ident[:, :])

        zeros4 = singles.tile([P, BL], dt.float32)
        nc.vector.memset(zeros4[:, :], 0.0)
        zeros4h = singles.tile([P, BL], dt.bfloat16)
        nc.vector.memset(zeros4h[:, :], 0.0)

        # ---- embedding gather (tokens on partitions) + transpose ----
        x_sb = singles.tile([P, BL, E], dt.float32)
        for m in range(BL):
            nc.gpsimd.indirect_dma_start(
                out=x_sb[:, m, :], out_offset=None,
                in_=emb_d[:, :],
                in_offset=IndirectOffsetOnAxis(ap=ids_sb[:, m:m + 1], axis=0),
            )
        xT = singles.tile([P, NTOK], dt.bfloat16)
        for m in range(BL):
            pst = ps_big.tile([P, P], dt.float32, tag="psac")
            nc.tensor.transpose(pst[:, :], x_sb[:, m, :], ident[:, :])
            nc.scalar.copy(xT[:, m * P:(m + 1) * P], pst[:, :])

        # ---- A/C buffers and hidden-state buffers ----
        a_sb = [big.tile([P, 4, NTOK], dt.float32, name=f"a{l}", tag=f"a{l}")
                for l in range(L)]
        c_sb = [big.tile([P, 4, NTOK], dt.float32, name=f"cc{l}", tag=f"cc{l}")
                for l in range(L)]
        hT = [big.tile([P, NTOK], dt.bfloat16, name=f"h{l}", tag=f"h{l}")
              for l in range(L)]

        def emit_ac_block(l, src, j):
            # A[:, k, jblk] = WxA[l,k].T @ src_blk + beta2 ; C likewise + bias
            blk = slice(j * TPB, (j + 1) * TPB)
            for k in range(4):
                psa = ps_big.tile([P, TPB], dt.float32, tag="psac")
                nc.tensor.matmul(psa[:, :], wxa_sb[:, l, k * P:(k + 1) * P],
                                 src[:, blk])
                nc.scalar.activation(a_sb[l][:, k, blk], psa[:, :],
                                     AF.Identity, bias=b2t_sb[:, l, k:k + 1])
                psc = ps_big.tile([P, TPB], dt.float32, tag="psac")
                nc.tensor.matmul(psc[:, :], wxc_sb[:, l, k * P:(k + 1) * P],
                                 src[:, blk])
                nc.vector.tensor_scalar_add(c_sb[l][:, k, blk], psc[:, :],
                                            bft_sb[:, l, k:k + 1])

        # per-layer recurrence state (APs)
        c_prev = [zeros4[:, :], zeros4[:, :]]
        h_prev = [zeros4h[:, :], zeros4h[:, :]]
        ps_g = [ps_g0, ps_g1]

        def emit_step(l, t):
            tb = slice(t * BL, (t + 1) * BL)
            psg = ps_g[l].tile([P, 4, BL], dt.float32)
            for k in range(4):
                nc.tensor.matmul(psg[:, k, :], wh_sb[:, l, k * P:(k + 1) * P],
                                 h_prev[l], start=(k == 0), stop=(k == 3),
                                 skip_group_check=True)
            g = rec.tile([P, 4, BL], dt.float32, tag=f"g{l}")
            nc.vector.tensor_tensor(g[:, :, :], psg[:, :, :],
                                    a_sb[l][:, :, tb], op=OP.mult)
            nc.vector.tensor_tensor(g[:, :, :], g[:, :, :],
                                    c_sb[l][:, :, tb], op=OP.add)
            cp = c_prev[l]
            if2 = rec.tile([P, 2, BL], dt.float32, tag=f"if{l}")
            nc.vector.scalar_tensor_tensor(
                if2[:, 0, :], cp, pep_sb[:, l, 0:1], g[:, 0, :],
                op0=OP.mult, op1=OP.add)
            nc.vector.scalar_tensor_tensor(
                if2[:, 1, :], cp, pep_sb[:, l, 1:2], g[:, 1, :],
                op0=OP.mult, op1=OP.add)
            sif = rec.tile([P, 2, BL], dt.float32, tag=f"sif{l}")
            nc.scalar.activation(sif[:, :, :], if2[:, :, :], AF.Sigmoid)
            tj = rec.tile([P, BL], dt.float32, tag=f"tj{l}")
            nc.scalar.activation(tj[:, :], g[:, 2, :], AF.Tanh)
            u = rec.tile([P, BL], dt.float32, tag=f"u{l}")
            nc.gpsimd.tensor_tensor(u[:, :], sif[:, 0, :], tj[:, :],
                                    op=OP.mult)
            v = rec.tile([P, BL], dt.float32, tag=f"v{l}")
            nc.vector.tensor_tensor(v[:, :], sif[:, 1, :], cp, op=OP.mult)
            cn = cpool.tile([P, BL], dt.float32, tag=f"c{l}")
            nc.vector.tensor_tensor(cn[:, :], u[:, :], v[:, :], op=OP.add)
            o2 = rec.tile([P, BL], dt.float32, tag=f"o2{l}")
            nc.vector.scalar_tensor_tensor(
                o2[:, :], cn[:, :], pep_sb[:, l, 2:3], g[:, 3, :],
                op0=OP.mult, op1=OP.add)
            so = rec.tile([P, BL], dt.float32, tag=f"so{l}")
            nc.scalar.activation(so[:, :], o2[:, :], AF.Sigmoid)
            tc_ = rec.tile([P, BL], dt.float32, tag=f"tc{l}")
            nc.scalar.activation(tc_[:, :], cn[:, :], AF.Tanh)
            nc.vector.tensor_tensor(hT[l][:, tb], so[:, :], tc_[:, :],
                                    op=OP.mult)
            c_prev[l] = cn[:, :]
            h_prev[l] = hT[l][:, tb]

        def emit_logits_ntile(k, n, eng):
            n0 = n * 512
            nn = 512 if n < NT_FULL else NT_LAST
            ps = ps_log.tile([P, 512], dt.float32)
            nc.tensor.matmul(ps[:, 0:nn], hT[1][:, k * TPB:(k + 1) * TPB],
                             swt_sb[:, n0:n0 + nn],
                             start=True, stop=not use_smax_bias)
            if use_smax_bias:
                nc.tensor.matmul(ps[:, 0:nn], ones1[:, :],
                                 smb_sb[:, n0:n0 + nn], start=False, stop=True)
            st = stage_p.tile([P, 512], dt.float32)
            if eng == 0:
                nc.vector.tensor_copy(st[:, 0:nn], ps[:, 0:nn])
            else:
                nc.scalar.copy(st[:, 0:nn], ps[:, 0:nn])
            nc.sync.dma_start(
                out=out_d[k * TPB:(k + 1) * TPB, n0:n0 + nn],
                in_=st[:, 0:nn])

        # layer-0 A/C for all tokens (x fully available)
        for j in range(NB):
            emit_ac_block(0, xT, j)

        # ---- pipelined recurrence + logits ----
        pending = []   # (mtile, ntile) logits work
        ne = 0
        for jj in range(NB + 1):
            for i in range(SPB):
                if jj < NB and stage >= 1:
                    emit_step(0, jj * SPB + i)
                if jj >= 1 and stage >= 2:
                    emit_step(1, (jj - 1) * SPB + i)
                for _ in range(2):
                    if ne < len(pending):
                        k, n = pending[ne]
                        emit_logits_ntile(k, n, ne % 2)
                        ne += 1
            if jj < NB and stage >= 2:
                emit_ac_block(1, hT[0], jj)
            if jj >= 1 and stage >= 3:
                pending.extend(((jj - 1, n) for n in range(N_NT)))
        while ne < len(pending):
            k, n = pending[ne]
            emit_logits_ntile(k, n, ne % 2)
            ne += 1

    nc.compile()
    return nc


def _prep_inputs(input_data, embedding, Wx, Wh, alpha, beta1, beta2, bias,
                 wi, wf, wo, softmax_w, softmax_b):
    f32 = np.float32
    input_data = np.asarray(input_data, np.int32)
    embedding = np.ascontiguousarray(np.asarray(embedding, f32))
    Wx = np.asarray(Wx, f32)
    Wh = np.asarray(Wh, f32)
    alpha = np.asarray(alpha, f32)
    beta1 = np.asarray(beta1, f32)
    beta2 = np.asarray(beta2, f32)
    bias = np.asarray(bias, f32)
    wi = np.asarray(wi, f32)
    wf = np.asarray(wf, f32)
    wo = np.asarray(wo, f32)
    softmax_w = np.asarray(softmax_w, f32)
    softmax_b = np.asarray(softmax_b, f32)

    gperm = [0, 2, 1, 3]   # reference order i,j,f,o -> device order i,f,j,o

    def permG(a):           # (..., G) -> gate chunks reordered
        r = a.reshape(*a.shape[:-1], 4, E)
        return np.ascontiguousarray(r[..., gperm, :].reshape(*a.shape))

    WxA = permG(Wx * alpha[:, None, :])          # (L, E, G)
    WxC = permG(Wx * beta1[:, None, :])
    Whp = permG(Wh)
    b2p = permG(beta2)                           # (L, G)
    bp = permG(bias).copy()
    bp[:, E:2 * E] += FORGET_BIAS                # f-chunk

    def to_elg(a):          # (L, E, G) -> (E, L, G) contiguous
        return np.ascontiguousarray(np.transpose(a, (1, 0, 2)))

    def to_plk(a):          # (L, G) -> (P, L, 4): partition=feature, per-gate
        return np.ascontiguousarray(
            np.transpose(a.reshape(L, 4, E), (2, 0, 1)))

    pep = np.ascontiguousarray(
        np.transpose(np.stack([wi, wf, wo], axis=1), (2, 0, 1)))  # (E, L, 3)

    swt = np.ascontiguousarray(softmax_w.T)      # (E, V)
    use_smax_bias = bool(np.any(softmax_b))

    import ml_dtypes
    bf16 = ml_dtypes.bfloat16
    common = {
        "emb": embedding,
        "wxa": to_elg(WxA).astype(bf16), "wxc": to_elg(WxC).astype(bf16),
        "wh": to_elg(Whp).astype(bf16),
        "b2t": to_plk(b2p), "bft": to_plk(bp), "pep": pep,
        "swt": swt.astype(bf16),
    }
    if use_smax_bias:
        common["smb"] = softmax_b.reshape(1, V)

    tok = np.arange(NTOK)
    tt_, ss_ = tok // BL, tok % BL
    in_maps = []
    for c in range(NCORES):
        flat = input_data[BL * c + ss_, tt_]               # token order t*BL+s
        ids_pm = np.ascontiguousarray(flat.reshape(BL, P).T.astype(np.int32))
        in_maps.append({"ids": ids_pm, **common})
    return in_maps, use_smax_bias


def _run(in_maps, use_smax_bias, trace=False, tmpdir=None):
    from concourse.bass_utils import run_bass_kernel_spmd
    key = use_smax_bias
    if key not in _cache:
        _cache[key] = _build(use_smax_bias)
    nc = _cache[key]
    return run_bass_kernel_spmd(nc, in_maps, core_ids=list(range(NCORES)),
                                trace=trace, tmpdir=tmpdir)


def kernel(**inputs):
    in_maps, use_smax_bias = _prep_inputs(**inputs)
    res = _run(in_maps, use_smax_bias, trace=False)
    # device rows are token order (t*BL + s); reference rows are s*T + t
    tok = np.arange(NTOK)
    row = (tok % BL) * T + tok // BL          # device row tok -> output row
    out = np.empty((B * T, V), np.float32)
    for c in range(NCORES):
        out[c * NTOK + row] = res.results[c]["out"]
    return out
